# revision 7
# baseline (speedup 1.0000x reference)
"""Trainium2 Bass kernel for nn_Encoder (sliding-window NSA + downsample + LN + pooled SA).

Contract: kernel(**inputs) takes FULL unsharded numpy inputs, shards batch B=64
across 8 NeuronCores (8 batches each), runs one SPMD Bass program via
run_bass_kernel_spmd, gathers the [64, 2048] output.

Key design points:
- All heavy matmuls run as float32r (fp32 data, fp22 multiply) at full PE rate.
- Layout transposes the PE needs (contraction dim on partitions) are done
  host-side in numpy before staging DRAM inputs.
- LN affine (ln_g/ln_b) is folded into the SA projection weights on the host;
  q/k biases are applied per-partition on PSUM->SBUF copies (softmax-invariant
  terms need no correction); v biases are folded via an extra rank-1 matmul
  (NSA) and a host-side output shift (SA).
- Sliding-window attention uses the full per-batch 64x64 gram matrix; the
  windowed softmax runs on a flattened [batch, 4096] layout where the window
  diagonal becomes a stride-65 access pattern.
"""

import numpy as np
from contextlib import ExitStack

import concourse.bass as bass
import concourse.tile as tile
from concourse import bacc, mybir

F32 = mybir.dt.float32
F32R = mybir.dt.float32r
AF = mybir.ActivationFunctionType
AX = mybir.AxisListType

B, N, D, NB = 64, 64, 2048, 8
W = N - NB + 1          # 57 windows
NCORES = 8
BL = B // NCORES        # 8 batches per core
M1 = BL * N             # 512 activation rows, phase 1
M2 = BL * 64            # 512 padded rows, phase 4 (57 real + 7 pad per batch)
KT = D // 128           # 16 contraction tiles
SCALE = 1.0 / float(np.sqrt(2048.0))
EPS = 1e-5


def _r(ap, s, **kw):
    return ap.rearrange(s, **kw)


def _proj_T(nc, wpool, pspool, out_tile, wT_dram, rhs_sb, bias_sb, m):
    """Transposed-output projection: out^T[n, m] in [128, 16*m] (col block nt).
    psum[n128, m] = sum_k wT[k128, n128].T @ rhs[k128, m]. Per-partition (per-n)
    bias is added on the PSUM->SBUF copy."""
    for half in range(2):
        pss = [pspool.tile([128, m], F32, tag=f"b{j}", name=f"psT{j}") for j in range(8)]
        for k in range(KT):
            wch = wpool.tile([128, 1024], F32R, tag="wch", name="wch")
            nc.sync.dma_start(
                wch[:], wT_dram[k * 128:(k + 1) * 128, half * 1024:(half + 1) * 1024])
            for j in range(8):
                nc.tensor.matmul(
                    pss[j][:],
                    wch[:, j * 128:(j + 1) * 128],
                    rhs_sb[:, k * m:(k + 1) * m],
                    start=(k == 0), stop=(k == KT - 1))
        for j in range(8):
            nt = half * 8 + j
            dst = out_tile[:, nt * m:(nt + 1) * m]
            if bias_sb is not None:
                nc.vector.tensor_scalar_add(dst, pss[j][:], bias_sb[:, nt:nt + 1])
            else:
                nc.vector.tensor_copy(dst, pss[j][:])


def _proj_nat(nc, wpool, pspool, out_tile, wT_dram, lhs_sb, m):
    """Natural-layout projection out[m, n]: out tile [128, (m//128)*D], column
    block mt holds rows mt*128.., free = n."""
    mt_cnt = m // 128
    for pair in range(2):
        pss = [pspool.tile([128, 512], F32, tag=f"b{j}", name=f"psN{j}") for j in range(8)]
        for k in range(KT):
            wch = wpool.tile([128, 1024], F32R, tag="wch", name="wch")
            nc.sync.dma_start(
                wch[:], wT_dram[k * 128:(k + 1) * 128, pair * 1024:(pair + 1) * 1024])
            for mt in range(mt_cnt):
                for ci in range(2):
                    nc.tensor.matmul(
                        pss[mt * 2 + ci][:],
                        lhs_sb[:, k * m + mt * 128: k * m + mt * 128 + 128],
                        wch[:, ci * 512:(ci + 1) * 512],
                        start=(k == 0), stop=(k == KT - 1))
        for mt in range(mt_cnt):
            for ci in range(2):
                ncol = pair * 1024 + ci * 512
                nc.vector.tensor_copy(
                    out_tile[:, mt * D + ncol: mt * D + ncol + 512],
                    pss[mt * 2 + ci][:])


def build_nc():
    nc = bacc.Bacc("TRN2", target_bir_lowering=False, debug=False,
                   num_devices=NCORES)

    def din(name, shape, dt=F32):
        return nc.dram_tensor(name, list(shape), dt, kind="ExternalInput").ap()

    xT_d = din("xT", (D, M1), F32R)
    xn_d = din("xn", (128, 4 * D), F32R)
    wqT_d = din("wqT", (D, D), F32R); wkT_d = din("wkT", (D, D), F32R); wvT_d = din("wvT", (D, D), F32R)
    w2qT_d = din("w2qT", (D, D), F32R); w2kT_d = din("w2kT", (D, D), F32R); w2vT_d = din("w2vT", (D, D), F32R)
    bq_d = din("bq_t", (128, KT)); bk_d = din("bk_t", (128, KT))
    b2q_d = din("b2q_t", (128, KT)); b2k_d = din("b2k_t", (128, KT))
    dswT_d = din("dswT", (N, W), F32R)
    dsb_d = din("dsb", (W, 1))
    ident_d = din("ident", (128, 128))
    bvaug_d = din("bv_aug", (1, D), F32R)
    onesw_d = din("onesw", (1, W), F32R)
    zer1_d = din("zer1", (128, 1), F32R)
    out_d = nc.dram_tensor("out", [BL, D], F32, kind="ExternalOutput").ap()

    with tile.TileContext(nc) as tc, ExitStack() as ctx:
        const = ctx.enter_context(tc.tile_pool(name="const", bufs=1))
        wpool = ctx.enter_context(tc.tile_pool(name="w", bufs=2))
        pspool = ctx.enter_context(tc.tile_pool(name="ps", bufs=1, space="PSUM"))
        dpool = ctx.enter_context(tc.tile_pool(name="scr", bufs=1, space="DRAM"))

        # per-lifetime pools, opened/closed in LIFO stack order per SBUF side
        sA = ExitStack(); pA = sA.enter_context(tc.tile_pool(name="pA", bufs=1))  # xT (left)
        sB = ExitStack(); pB = sB.enter_context(tc.tile_pool(name="pB", bufs=1))  # qT,kT (left)
        sC2 = ExitStack(); pC2 = sC2.enter_context(
            tc.tile_pool(name="pC2", bufs=1, side="right"))  # a_sb (right)

        # ---- constants ----
        bq_sb = const.tile([128, KT], F32); nc.sync.dma_start(bq_sb[:], bq_d)
        bk_sb = const.tile([128, KT], F32); nc.sync.dma_start(bk_sb[:], bk_d)
        b2q_sb = const.tile([128, KT], F32); nc.sync.dma_start(b2q_sb[:], b2q_d)
        b2k_sb = const.tile([128, KT], F32); nc.sync.dma_start(b2k_sb[:], b2k_d)
        dswT_sb = const.tile([128, W], F32R)
        nc.sync.dma_start(dswT_sb[:][0:N, :], dswT_d)
        nc.sync.dma_start(dswT_sb[:][N:128, :], dswT_d)
        dsb_sb = const.tile([W, 1], F32); nc.sync.dma_start(dsb_sb[:], dsb_d)
        ident_sb = const.tile([128, 128], F32); nc.sync.dma_start(ident_sb[:], ident_d)
        bvaug_sb = const.tile([1, D], F32R); nc.sync.dma_start(bvaug_sb[:], bvaug_d)
        ones_row = const.tile([1, W], F32R); nc.sync.dma_start(ones_row[:], onesw_d)
        zer_sb = const.tile([128, 1], F32R); nc.sync.dma_start(zer_sb[:], zer1_d)
        epscol = const.tile([128, 1], F32); nc.vector.memset(epscol[:], EPS)
        onescol = const.tile([W, 1], F32); nc.vector.memset(onescol[:], 1.0)

        # ---- load x^T ----
        xT_sb = pA.tile([128, KT * M1], F32R)
        nc.sync.dma_start(
            _r(xT_sb[:], "p (k m) -> p k m", k=KT),
            _r(xT_d, "(k p) m -> p k m", p=128))

        # ---- NSA Q^T, K^T (biased) ----
        qT_sb = pB.tile([128, KT * M1], F32, tag="qT")
        kT_sb = pB.tile([128, KT * M1], F32, tag="kT")
        _proj_T(nc, wpool, pspool, qT_sb[:], wqT_d, xT_sb[:], bq_sb[:], M1)
        _proj_T(nc, wpool, pspool, kT_sb[:], wkT_d, xT_sb[:], bk_sb[:], M1)

        # ---- per-batch gram G[t, t'] ----
        sC = ExitStack(); pC = sC.enter_context(tc.tile_pool(name="pC", bufs=1))
        g_sb = pC.tile([N, BL * N], F32, tag="g")
        for b in range(BL):
            gps = pspool.tile([N, N], F32, tag=f"b{b % 2}", name=f"gps{b}")
            for k in range(KT):
                nc.tensor.matmul(
                    gps[:],
                    qT_sb[:, k * M1 + b * N: k * M1 + b * N + N],
                    kT_sb[:, k * M1 + b * N: k * M1 + b * N + N],
                    start=(k == 0), stop=(k == KT - 1))
            nc.vector.tensor_copy(g_sb[:, b * N:(b + 1) * N], gps[:])

        # ---- flatten G to [b, t*64+t'] via DRAM ----
        g_dram = dpool.tile([N, BL * N], F32)
        nc.sync.dma_start(g_dram[:], g_sb[:])
        gflat = pC.tile([BL, N * N], F32, tag="gflat")
        nc.sync.dma_start(
            _r(gflat[:], "p (t u) -> p t u", t=N),
            bass.AP(g_dram[:].tensor, 0, [[N, BL], [BL * N, N], [1, N]]))

        # ---- windowed softmax on [batch, ...] layout ----
        E_sb = pC.tile([BL, W * NB * NB], F32, tag="E")
        win = bass.AP(gflat[:].tensor, gflat[:].offset,
                      [[N * N, BL], [N + 1, W], [N, NB], [1, NB]])
        nc.scalar.activation(
            _r(E_sb[:], "p (w i j) -> p w i j", i=NB, j=NB), win, AF.Exp, scale=SCALE)
        Z_sb = pC.tile([BL, W * NB], F32, tag="Z")
        nc.vector.reduce_sum(
            Z_sb[:], _r(E_sb[:], "p (a j) -> p a j", j=NB), axis=AX.X)
        R_sb = pC.tile([BL, W * NB], F32, tag="R")
        nc.vector.reciprocal(R_sb[:], Z_sb[:])
        Ev = _r(E_sb[:], "p (w i j) -> p w i j", i=NB, j=NB)
        Rv = _r(R_sb[:], "p (w i) -> p w i", i=NB).unsqueeze(3).broadcast_to(
            [BL, W, NB, NB])
        nc.vector.tensor_mul(Ev, Ev, Rv)
        s_sb = pC.tile([BL, W * NB], F32, tag="s")
        nc.vector.reduce_sum(
            s_sb[:], _r(E_sb[:], "p (w i j) -> p w j i", i=NB, j=NB), axis=AX.X)

        # ---- banded A via DRAM scatter: aT[(w*8+b), t] = s[b, w, t-w] ----
        s_dram = dpool.tile([BL, W * NB], F32)
        nc.sync.dma_start(s_dram[:], s_sb[:])
        aT_dram = dpool.tile([W * NB + NB, N], F32)  # 464 rows
        zrows = pC.tile([128, N], F32, tag="zrows")
        nc.vector.memset(zrows[:], 0.0)
        for kb in range(4):
            r0 = kb * 128
            r1 = min(r0 + 128, W * NB + NB)
            nc.sync.dma_start(aT_dram[:][r0:r1, :], zrows[:][0:r1 - r0, :])
        nc.sync.dma_start(
            bass.AP(aT_dram[:].tensor, 0, [[N, BL], [NB * N + 1, W], [1, NB]]),
            bass.AP(s_dram[:].tensor, 0, [[W * NB, BL], [NB, W], [1, NB]]))
        aT_sb = pC.tile([128, 4 * N], F32, tag="aT")
        for kb in range(4):
            r0 = kb * 128
            r1 = min(r0 + 128, W * NB)
            nc.sync.dma_start(aT_sb[:][0:r1 - r0, kb * N:(kb + 1) * N],
                              aT_dram[:][r0:r1, :])
        a_sb = pC2.tile([128, W * NB], F32R)
        for kb in range(4):
            r0 = kb * 128
            r1 = min(r0 + 128, W * NB)
            rr = r1 - r0
            tp = pspool.tile([N, 128], F32, tag=f"b{2 + kb % 2}", name=f"tpa{kb}")
            nc.tensor.transpose(tp[:][:, 0:rr],
                                aT_sb[:][0:rr, kb * N:(kb + 1) * N],
                                ident_sb[:][0:rr, 0:rr])
            nc.vector.tensor_copy(a_sb[:][0:N, r0:r1], tp[:][:, 0:rr])
            nc.vector.tensor_copy(a_sb[:][N:128, r0:r1], tp[:][:, 0:rr])
        sC.close(); sB.close()

        # ---- V natural; x natural ----
        sD = ExitStack(); pD = sD.enter_context(tc.tile_pool(name="pD", bufs=1))
        v_sb = pD.tile([128, 4 * D], F32R, tag="v")
        _proj_nat(nc, wpool, pspool, v_sb[:], wvT_d, xT_sb[:], M1)
        xn_sb = pD.tile([128, 4 * D], F32R, tag="xn")
        nc.sync.dma_start(xn_sb[:], xn_d)

        # ---- nsa + downsample fused in PSUM; LN per batch-pair ----
        sE = ExitStack(); pE = sE.enter_context(tc.tile_pool(name="pE", bufs=1))
        h_tiles = [pE.tile([128, D], F32, tag=f"h{p}", name=f"h{p}") for p in range(4)]
        acc = pE.tile([128, 4], F32, tag="acc")
        stat = pE.tile([128, 8], F32, tag="stat")
        sq_scr = pE.tile([128, D], F32, tag="sqscr")
        nc.vector.memset(acc[:], 0.0)
        for p in range(4):
            ht = h_tiles[p][:]
            nc.gpsimd.memset(ht, 0.0)
            for bi in range(2):
                b = 2 * p + bi
                up = bi * 64
                for c in range(4):
                    hp = pspool.tile([W, 512], F32, tag=f"b{(bi * 4 + c) % 8}", name=f"hp{p}_{bi}_{c}")
                    av = _r(a_sb[:][up:up + 64, :], "t (w e) -> t w e", e=NB)[:, :, b]
                    nc.tensor.matmul(
                        hp[:], av,
                        v_sb[:][up:up + 64, p * D + c * 512: p * D + (c + 1) * 512],
                        start=True, stop=False)
                    nc.tensor.matmul(
                        hp[:], ones_row[:],
                        bvaug_sb[:][:, c * 512:(c + 1) * 512],
                        start=False, stop=False)
                    nc.tensor.matmul(
                        hp[:], dswT_sb[:][up:up + 64, :],
                        xn_sb[:][up:up + 64, p * D + c * 512: p * D + (c + 1) * 512],
                        start=False, stop=True)
                    nc.scalar.activation(
                        ht[up:up + W, c * 512:(c + 1) * 512], hp[:],
                        AF.Identity, bias=dsb_sb[:],
                        accum_out=acc[:][up:up + W, c:c + 1])
            # LN stats for this pair
            nc.vector.reduce_sum(stat[:][:, 0:1], acc[:], axis=AX.X)
            nc.vector.tensor_scalar_mul(stat[:][:, 1:2], stat[:][:, 0:1], -1.0 / D)
            nc.scalar.activation(ht, ht, AF.Identity, bias=stat[:][:, 1:2])
            nc.scalar.activation(sq_scr[:], ht, AF.Square, accum_out=stat[:][:, 2:3])
            nc.scalar.activation(stat[:][:, 3:4], stat[:][:, 2:3], AF.Sqrt,
                                 scale=1.0 / D, bias=epscol[:])
            nc.vector.reciprocal(stat[:][:, 4:5], stat[:][:, 3:4])
            nc.scalar.activation(ht, ht, AF.Identity, scale=stat[:][:, 4:5])
        sC2.close()

        # ---- transpose z -> hnT [128, 16*512] (pad cols zero) ----
        sF = ExitStack(); pF = sF.enter_context(
            tc.tile_pool(name="pF", bufs=1, side="right"))
        hnT_sb = pF.tile([128, KT * M2], F32R)
        nc.vector.tensor_copy(
            bass.AP(hnT_sb[:].tensor, hnT_sb[:].offset + W,
                    [[KT * M2, 128], [M2, KT], [64, NB], [1, 64 - W]]),
            zer_sb[:].unsqueeze(1).unsqueeze(1).broadcast_to([128, KT, NB, 64 - W]))
        for p in range(4):
            for dt in range(KT):
                tp = pspool.tile([128, 128], F32, tag=f"b{dt % 4}", name=f"tpz{p}_{dt}")
                nc.tensor.transpose(tp[:], h_tiles[p][:][:, dt * 128:(dt + 1) * 128],
                                    ident_sb[:])
                nc.vector.tensor_copy(
                    hnT_sb[:][:, dt * M2 + (2 * p) * 64: dt * M2 + (2 * p) * 64 + W],
                    tp[:][:, 0:W])
                nc.vector.tensor_copy(
                    hnT_sb[:][:, dt * M2 + (2 * p + 1) * 64: dt * M2 + (2 * p + 1) * 64 + W],
                    tp[:][:, 64:64 + W])
        sE.close(); sD.close(); sA.close()

        # ---- SA projections ----
        sG = ExitStack(); pG = sG.enter_context(tc.tile_pool(name="pG", bufs=1))
        q2T_sb = pG.tile([128, KT * M2], F32, tag="q2T")
        k2T_sb = pG.tile([128, KT * M2], F32, tag="k2T")
        _proj_T(nc, wpool, pspool, q2T_sb[:], w2qT_d, hnT_sb[:], b2q_sb[:], M2)
        _proj_T(nc, wpool, pspool, k2T_sb[:], w2kT_d, hnT_sb[:], b2k_sb[:], M2)
        sH = ExitStack(); pH = sH.enter_context(tc.tile_pool(name="pH", bufs=1))
        v2_sb = pH.tile([128, 4 * D], F32R)
        _proj_nat(nc, wpool, pspool, v2_sb[:], w2vT_d, hnT_sb[:], M2)
        sF.close()

        # ---- pooled attention ----
        sI = ExitStack(); pI = sI.enter_context(tc.tile_pool(name="pI", bufs=1))
        s2T_sb = pI.tile([128, BL], F32R, tag="s2T")
        nc.vector.tensor_copy(s2T_sb[:], zer_sb[:].broadcast_to([128, BL]))
        z2_sb = pI.tile([W, 2], F32, tag="z2")
        for b in range(BL):
            g2 = pspool.tile([64, 64], F32, tag=f"b{b % 2}", name=f"g2_{b}")
            for k in range(KT):
                nc.tensor.matmul(
                    g2[:],
                    q2T_sb[:, k * M2 + b * 64: k * M2 + b * 64 + 64],
                    k2T_sb[:, k * M2 + b * 64: k * M2 + b * 64 + 64],
                    start=(k == 0), stop=(k == KT - 1))
            e2 = pI.tile([64, 64], F32, tag=f"e2_{b % 2}", name=f"e2_{b}")
            nc.scalar.activation(e2[:], g2[:], AF.Exp, scale=SCALE)
            nc.vector.memset(e2[:][0:W, W:64], 0.0)
            zc = z2_sb[:][:, b % 2: b % 2 + 1]
            nc.vector.reduce_sum(zc, e2[:][0:W, :], axis=AX.X)
            nc.vector.reciprocal(zc, zc)
            nc.vector.tensor_scalar_mul(e2[:][0:W, :], e2[:][0:W, :], zc)
            s2p = pspool.tile([64, 1], F32, tag=f"b{2 + b % 2}", name=f"s2p{b}")
            nc.tensor.matmul(s2p[:], e2[:][0:W, :],
                             onescol[:], start=True, stop=True)
            nc.vector.tensor_copy(
                s2T_sb[:][(b % 2) * 64:(b % 2) * 64 + 64, b:b + 1], s2p[:])
        for p in range(4):
            outp = pI.tile([2, D], F32, tag=f"op{p % 2}", name=f"outp{p}")
            for c in range(4):
                op = pspool.tile([2, 512], F32, tag=f"b{4 + c}", name=f"ops{p}_{c}")
                nc.tensor.matmul(
                    op[:], s2T_sb[:][:, 2 * p: 2 * p + 2],
                    v2_sb[:][:, p * D + c * 512: p * D + (c + 1) * 512],
                    start=True, stop=True)
                nc.scalar.activation(outp[:][:, c * 512:(c + 1) * 512], op[:],
                                     AF.Identity)
            nc.sync.dma_start(out_d[2 * p: 2 * p + 2, :], outp[:])
        sF.close()
        sI.close(); sH.close(); sG.close()

    nc.compile()
    return nc


_CACHE = {}


def _host_prep(inputs):
    f32 = lambda x: np.ascontiguousarray(np.asarray(x, dtype=np.float32))
    fc = f32(inputs["fc_feats"])
    ln_g = f32(inputs["ln_g"]); ln_b = f32(inputs["ln_b"])
    T = lambda w: np.ascontiguousarray(np.asarray(w, dtype=np.float32).T)
    t16 = lambda v: np.ascontiguousarray(
        np.asarray(v, dtype=np.float32).reshape(KT, 128).T)

    sa_wq = f32(inputs["sa_wq"]); sa_wk = f32(inputs["sa_wk"]); sa_wv = f32(inputs["sa_wv"])
    w2q = sa_wq * ln_g[None, :]
    w2k = sa_wk * ln_g[None, :]
    w2v = sa_wv * ln_g[None, :]
    b2q = f32(inputs["sa_bq"]) + sa_wq @ ln_b
    b2k = f32(inputs["sa_bk"]) + sa_wk @ ln_b
    b2v = f32(inputs["sa_bv"]) + sa_wv @ ln_b

    common = {
        "wqT": T(inputs["nsa_wq"]), "wkT": T(inputs["nsa_wk"]), "wvT": T(inputs["nsa_wv"]),
        "w2qT": T(w2q), "w2kT": T(w2k), "w2vT": T(w2v),
        "bq_t": t16(inputs["nsa_bq"]), "bk_t": t16(inputs["nsa_bk"]),
        "b2q_t": t16(b2q), "b2k_t": t16(b2k),
        "dswT": T(inputs["ds_w"]),
        "dsb": f32(inputs["ds_b"]).reshape(W, 1),
        "ident": np.eye(128, dtype=np.float32),
        "bv_aug": (8.0 * f32(inputs["nsa_bv"])).reshape(1, D),
        "onesw": np.ones((1, W), dtype=np.float32),
        "zer1": np.zeros((128, 1), dtype=np.float32),
    }
    in_maps = []
    for c in range(NCORES):
        fcc = fc[c * BL:(c + 1) * BL]
        xT = np.ascontiguousarray(fcc.reshape(M1, D).T)
        xn = np.zeros((128, 4 * D), dtype=np.float32)
        for b in range(BL):
            xn[(b % 2) * 64:(b % 2) * 64 + 64, (b // 2) * D:(b // 2 + 1) * D] = fcc[b]
        m = dict(common)
        m["xT"] = xT
        m["xn"] = xn
        in_maps.append(m)
    return in_maps, b2v


def kernel(**inputs):
    from concourse import bass_utils
    if "nc" not in _CACHE:
        _CACHE["nc"] = build_nc()
    nc = _CACHE["nc"]
    in_maps, b2v = _host_prep(inputs)
    res = bass_utils.run_bass_kernel_spmd(nc, in_maps, core_ids=list(range(NCORES)))
    out = np.concatenate([r["out"] for r in res.results], axis=0)
    out = out + (float(W) * b2v)[None, :]
    return out.astype(np.float32)


# revision 15
# speedup vs baseline: 5.6925x; 5.6925x over previous
"""Trainium2 Bass kernel for nn_Encoder (sliding-window NSA + downsample + LN + pooled SA).

Contract: kernel(**inputs) takes FULL unsharded numpy inputs, shards batch B=64
across 8 NeuronCores (8 batches each), runs one SPMD Bass program via
run_bass_kernel_spmd, gathers the [64, 2048] output.

Key design points:
- All heavy matmuls run as float32r (fp32 data, fp22 multiply) at full PE rate.
- Layout transposes the PE needs (contraction dim on partitions) are done
  host-side in numpy before staging DRAM inputs.
- LN affine (ln_g/ln_b) is folded into the SA projection weights on the host;
  q/k biases are applied per-partition on PSUM->SBUF copies (softmax-invariant
  terms need no correction); v biases are folded via an extra rank-1 matmul
  (NSA) and a host-side output shift (SA).
- Sliding-window attention uses the full per-batch 64x64 gram matrix; the
  windowed softmax runs on a flattened [batch, 4096] layout where the window
  diagonal becomes a stride-65 access pattern.
"""

import numpy as np
from contextlib import ExitStack




import concourse.bass as bass
import concourse.tile as tile
from concourse import bacc, mybir

F32 = mybir.dt.float32
F32R = mybir.dt.float32r
AF = mybir.ActivationFunctionType
AX = mybir.AxisListType

B, N, D, NB = 64, 64, 2048, 8
W = N - NB + 1          # 57 windows
NCORES = 8
BL = B // NCORES        # 8 batches per core
M1 = BL * N             # 512 activation rows, phase 1
M2 = BL * 64            # 512 padded rows, phase 4 (57 real + 7 pad per batch)
KT = D // 128           # 16 contraction tiles
SCALE = 1.0 / float(np.sqrt(2048.0))
EPS = 1e-5


def _r(ap, s, **kw):
    return ap.rearrange(s, **kw)


def _proj_T(nc, wpool, pspool, out_tile, wT_dram, rhs_sb, bias_sb, m, k_hook=None):
    """Transposed-output projection: out^T[n, m] in [128, 16*m] (col block nt).
    psum[n128, m] = sum_k wT[k128, n128].T @ rhs[k128, m]. Per-partition (per-n)
    bias is added on the PSUM->SBUF copy."""
    for half in range(2):
        pss = [pspool.tile([128, m], F32, tag=f"b{j}", name=f"psT{j}") for j in range(8)]
        for k in range(KT):
            if k_hook is not None and half == 0:
                k_hook(k)
            wch = wpool.tile([128, 1024], F32R, tag="wch", name="wch")
            nc.sync.dma_start(
                wch[:], wT_dram[k * 128:(k + 1) * 128, half * 1024:(half + 1) * 1024])
            for j in range(8):
                nc.tensor.matmul(
                    pss[j][:],
                    wch[:, j * 128:(j + 1) * 128],
                    rhs_sb[:, k * m:(k + 1) * m],
                    start=(k == 0), stop=(k == KT - 1))
        for j in range(8):
            nt = half * 8 + j
            dst = out_tile[:, nt * m:(nt + 1) * m]
            if bias_sb is not None:
                nc.vector.tensor_scalar_add(dst, pss[j][:], bias_sb[:, nt:nt + 1])
            else:
                nc.vector.tensor_copy(dst, pss[j][:])


def _proj_nat(nc, wpool, pspool, out_tile, wT_dram, lhs_sb, m):
    """Natural-layout projection out[m, n]: out tile [128, (m//128)*D], column
    block mt holds rows mt*128.., free = n."""
    mt_cnt = m // 128
    for pair in range(2):
        pss = [pspool.tile([128, 512], F32, tag=f"b{j}", name=f"psN{j}") for j in range(8)]
        for k in range(KT):
            wch = wpool.tile([128, 1024], F32R, tag="wch", name="wch")
            nc.sync.dma_start(
                wch[:], wT_dram[k * 128:(k + 1) * 128, pair * 1024:(pair + 1) * 1024])
            for mt in range(mt_cnt):
                for ci in range(2):
                    nc.tensor.matmul(
                        pss[mt * 2 + ci][:],
                        lhs_sb[:, k * m + mt * 128: k * m + mt * 128 + 128],
                        wch[:, ci * 512:(ci + 1) * 512],
                        start=(k == 0), stop=(k == KT - 1))
        for mt in range(mt_cnt):
            for ci in range(2):
                ncol = pair * 1024 + ci * 512
                nc.vector.tensor_copy(
                    out_tile[:, mt * D + ncol: mt * D + ncol + 512],
                    pss[mt * 2 + ci][:])


def build_nc():
    nc = bacc.Bacc("TRN2", target_bir_lowering=False, debug=False,
                   num_devices=NCORES)

    def din(name, shape, dt=F32):
        return nc.dram_tensor(name, list(shape), dt, kind="ExternalInput").ap()

    xT_d = din("xT", (D, M1), F32R)
    xn_d = din("xn", (128, 4 * D), F32R)
    wqT_d = din("wqT", (D, D), F32R); wkT_d = din("wkT", (D, D), F32R); wvT_d = din("wvT", (D, D), F32R)
    w2qT_d = din("w2qT", (D, D), F32R); w2kT_d = din("w2kT", (D, D), F32R); w2vT_d = din("w2vT", (D, D), F32R)
    bq_d = din("bq_t", (128, KT)); bk_d = din("bk_t", (128, KT))
    b2q_d = din("b2q_t", (128, KT)); b2k_d = din("b2k_t", (128, KT))
    dswT_d = din("dswT", (N, W), F32R)
    dsb_d = din("dsb", (W, 1))
    ident_d = din("ident", (128, 128))
    bvaug_d = din("bv_aug", (1, D), F32R)
    onesw_d = din("onesw", (1, W), F32R)
    zer1_d = din("zer1", (128, 1), F32R)
    out_d = nc.dram_tensor("out", [BL, D], F32, kind="ExternalOutput").ap()

    with tile.TileContext(nc) as tc, ExitStack() as ctx:
        const = ctx.enter_context(tc.tile_pool(name="const", bufs=1))
        wpool = ctx.enter_context(tc.tile_pool(name="w", bufs=5))
        dpool = ctx.enter_context(tc.tile_pool(name="scr", bufs=1, space="DRAM"))
        sPS1 = ExitStack()
        pspool = sPS1.enter_context(tc.tile_pool(name="ps1", bufs=1, space="PSUM"))

        # ---- constants ----
        bq_sb = const.tile([128, KT], F32); nc.sync.dma_start(bq_sb[:], bq_d)
        bk_sb = const.tile([128, KT], F32); nc.sync.dma_start(bk_sb[:], bk_d)
        b2q_sb = const.tile([128, KT], F32); nc.sync.dma_start(b2q_sb[:], b2q_d)
        b2k_sb = const.tile([128, KT], F32); nc.sync.dma_start(b2k_sb[:], b2k_d)
        dswT_sb = const.tile([128, W], F32R)
        nc.sync.dma_start(dswT_sb[:][0:N, :], dswT_d)
        nc.sync.dma_start(dswT_sb[:][N:128, :], dswT_d)
        dsb_sb = const.tile([W, 1], F32); nc.sync.dma_start(dsb_sb[:], dsb_d)
        ident_sb = const.tile([128, 128], F32); nc.sync.dma_start(ident_sb[:], ident_d)
        bvaug_sb = const.tile([1, D], F32R); nc.sync.dma_start(bvaug_sb[:], bvaug_d)
        ones_row = const.tile([1, W], F32R); nc.sync.dma_start(ones_row[:], onesw_d)
        zer_sb = const.tile([128, 1], F32R); nc.sync.dma_start(zer_sb[:], zer1_d)
        epscol = const.tile([128, 1], F32); nc.vector.memset(epscol[:], EPS)
        onescol = const.tile([W, 1], F32); nc.vector.memset(onescol[:], 1.0)

        # left-side pools (strict LIFO): A(xT/xn) B(qkT) D(v) C(softmax)
        sA = ExitStack(); pA = sA.enter_context(tc.tile_pool(name="pA", bufs=1))
        xT_sb = pA.tile([128, KT * M1], F32R, tag="xT")

        def _xT_load(k):
            nc.sync.dma_start(xT_sb[:, k * M1:(k + 1) * M1],
                              xT_d[k * 128:(k + 1) * 128, :])

        # ---- NSA Q/K projections ----
        sD = ExitStack(); pD = sD.enter_context(tc.tile_pool(name="pD", bufs=1))
        sB = ExitStack(); pB = sB.enter_context(tc.tile_pool(name="pB", bufs=1))
        qT_sb = pB.tile([128, KT * M1], F32, tag="qT")
        kT_sb = pB.tile([128, KT * M1], F32, tag="kT")
        _proj_T(nc, wpool, pspool, qT_sb[:], wqT_d, xT_sb[:], bq_sb[:], M1,
                k_hook=_xT_load)
        _proj_T(nc, wpool, pspool, kT_sb[:], wkT_d, xT_sb[:], bk_sb[:], M1)

        # ---- per-batch gram G[t, t'] ----
        sC = ExitStack(); pC = sC.enter_context(tc.tile_pool(name="pC", bufs=1))
        g_sb = pC.tile([N, BL * N], F32, tag="g")
        for b in range(BL):
            gps = pspool.tile([N, N], F32, tag=f"b{b % 2}", name=f"gps{b}")
            for k in range(KT):
                nc.tensor.matmul(
                    gps[:],
                    qT_sb[:, k * M1 + b * N: k * M1 + b * N + N],
                    kT_sb[:, k * M1 + b * N: k * M1 + b * N + N],
                    start=(k == 0), stop=(k == KT - 1))
            nc.vector.tensor_copy(g_sb[:, b * N:(b + 1) * N], gps[:])

        # ---- flatten G to [b, t*64+t'] via DRAM ----
        g_dram = dpool.tile([N, BL * N], F32)
        nc.sync.dma_start(g_dram[:], g_sb[:])
        gflat = pC.tile([BL, N * N], F32, tag="gflat")
        nc.sync.dma_start(
            _r(gflat[:], "p (t u) -> p t u", t=N),
            bass.AP(g_dram[:].tensor, 0, [[N, BL], [BL * N, N], [1, N]]))

        # ---- windowed softmax on [batch, ...] layout ----
        E_sb = pC.tile([BL, W * NB * NB], F32, tag="E")
        win = bass.AP(gflat[:].tensor, gflat[:].offset,
                      [[N * N, BL], [N + 1, W], [N, NB], [1, NB]])
        nc.scalar.activation(
            _r(E_sb[:], "p (w i j) -> p w i j", i=NB, j=NB), win, AF.Exp, scale=SCALE)
        Z_sb = pC.tile([BL, W * NB], F32, tag="Z")
        nc.vector.reduce_sum(
            Z_sb[:], _r(E_sb[:], "p (a j) -> p a j", j=NB), axis=AX.X)
        R_sb = pC.tile([BL, W * NB], F32, tag="R")
        nc.vector.reciprocal(R_sb[:], Z_sb[:])
        Ev = _r(E_sb[:], "p (w i j) -> p w i j", i=NB, j=NB)
        Rv = _r(R_sb[:], "p (w i) -> p w i", i=NB).unsqueeze(3).broadcast_to(
            [BL, W, NB, NB])
        nc.vector.tensor_mul(Ev, Ev, Rv)
        s_sb = pC.tile([BL, W * NB], F32, tag="s")
        nc.vector.reduce_sum(
            s_sb[:], _r(E_sb[:], "p (w i j) -> p w j i", i=NB, j=NB), axis=AX.X)

        # ---- banded A via DRAM scatter: aT[(w*8+b), t] = s[b, w, t-w] ----
        s_dram = dpool.tile([BL, W * NB], F32)
        nc.sync.dma_start(s_dram[:], s_sb[:])
        aT_dram = dpool.tile([W * NB + NB, N], F32)
        zrows = pC.tile([128, N], F32, tag="zrows")
        nc.vector.memset(zrows[:], 0.0)
        for kb in range(4):
            r0 = kb * 128
            r1 = min(r0 + 128, W * NB + NB)
            nc.sync.dma_start(aT_dram[:][r0:r1, :], zrows[:][0:r1 - r0, :])
        nc.sync.dma_start(
            bass.AP(aT_dram[:].tensor, 0, [[N, BL], [NB * N + 1, W], [1, NB]]),
            bass.AP(s_dram[:].tensor, 0, [[W * NB, BL], [NB, W], [1, NB]]))
        aT_sb = pC.tile([128, 4 * N], F32, tag="aT")
        for kb in range(4):
            r0 = kb * 128
            r1 = min(r0 + 128, W * NB)
            nc.sync.dma_start(aT_sb[:][0:r1 - r0, kb * N:(kb + 1) * N],
                              aT_dram[:][r0:r1, :])
        # ---- V projection (PE work overlapping the softmax/scatter chain) ----
        v_sb = pD.tile([128, 4 * D], F32R, tag="v")
        _proj_nat(nc, wpool, pspool, v_sb[:], wvT_d, xT_sb[:], M1)
        # xn reuses the xT slot (WAR dep on the projections' last read)
        xn_sb = pA.tile([128, 4 * D], F32R, tag="xT", name="xn_sb")
        nc.sync.dma_start(xn_sb[:], xn_d)

        sC2 = ExitStack(); pC2 = sC2.enter_context(
            tc.tile_pool(name="pC2", bufs=1, side="right"))
        a_sb = pC2.tile([128, W * NB], F32R)
        for kb in range(4):
            r0 = kb * 128
            r1 = min(r0 + 128, W * NB)
            rr = r1 - r0
            tp = pspool.tile([N, 128], F32, tag=f"b{2 + kb % 2}", name=f"tpa{kb}")
            nc.tensor.transpose(tp[:][:, 0:rr],
                                aT_sb[:][0:rr, kb * N:(kb + 1) * N],
                                ident_sb[:][0:rr, 0:rr])
            nc.vector.tensor_copy(a_sb[:][0:N, r0:r1], tp[:][:, 0:rr])
            nc.vector.tensor_copy(a_sb[:][N:128, r0:r1], tp[:][:, 0:rr])
        sC.close(); sB.close()

        # ---- nsa + downsample fused in PSUM; LN per batch-pair ----
        sPS1.close()
        sPS2 = ExitStack()
        pspool = sPS2.enter_context(tc.tile_pool(name="ps2", bufs=1, space="PSUM"))
        sE = ExitStack(); pE = sE.enter_context(tc.tile_pool(name="pE", bufs=1))
        h_tiles = [pE.tile([128, D], F32, tag=f"h{p}", name=f"h{p}") for p in range(4)]
        acc = pE.tile([128, 2], F32, tag="acc")
        acc2 = pE.tile([128, 2], F32, tag="acc2")
        stat = pE.tile([128, 8], F32, tag="stat")
        sq_scr = pE.tile([128, D], F32, tag="sqscr")
        for p in range(4):
            ht = h_tiles[p][:]
            for bi in range(2):
                b = 2 * p + bi
                up = bi * 64
                hp = pspool.tile([W, 2048], F32, tag=f"hp{bi}", name=f"hp{p}_{bi}")
                av = _r(a_sb[:][up:up + 64, :], "t (w e) -> t w e", e=NB)[:, :, b]
                for c in range(4):
                    cs = slice(c * 512, (c + 1) * 512)
                    nc.tensor.matmul(
                        hp[:][:, cs], av,
                        v_sb[:][up:up + 64, p * D + c * 512: p * D + (c + 1) * 512],
                        start=True, stop=False)
                    nc.tensor.matmul(
                        hp[:][:, cs], ones_row[:],
                        bvaug_sb[:][:, c * 512:(c + 1) * 512],
                        start=False, stop=False)
                    nc.tensor.matmul(
                        hp[:][:, cs], dswT_sb[:][up:up + 64, :],
                        xn_sb[:][up:up + 64, p * D + c * 512: p * D + (c + 1) * 512],
                        start=False, stop=True)
                nc.scalar.activation(
                    ht[up:up + W, :], hp[:],
                    AF.Identity, bias=dsb_sb[:],
                    accum_out=acc[:][up:up + W, bi:bi + 1])
                nc.vector.scalar_tensor_tensor(
                    sq_scr[:][up:up + W, :], ht[up:up + W, :], 1.0, ht[up:up + W, :],
                    op0=mybir.AluOpType.mult, op1=mybir.AluOpType.mult,
                    accum_out=acc2[:][up:up + W, bi:bi + 1])
            # LN stats for this pair: var = E[h^2] - mu^2
            for up in (0, 64):
                rows = slice(up, up + W)
                bi = up // 64
                nc.vector.tensor_scalar_mul(stat[:][rows, 1:2], acc[:][rows, bi:bi + 1], 1.0 / D)
                nc.vector.tensor_copy(stat[:][rows, 2:3], acc2[:][rows, bi:bi + 1])
                nc.vector.tensor_mul(stat[:][rows, 3:4], stat[:][rows, 1:2], stat[:][rows, 1:2])
                nc.vector.scalar_tensor_tensor(
                    stat[:][rows, 4:5], stat[:][rows, 2:3], 1.0 / D, stat[:][rows, 3:4],
                    op0=mybir.AluOpType.mult, op1=mybir.AluOpType.subtract)
                nc.scalar.activation(stat[:][rows, 5:6], stat[:][rows, 4:5], AF.Sqrt,
                                     bias=epscol[:][rows, :])
                nc.vector.reciprocal(stat[:][rows, 6:7], stat[:][rows, 5:6])
                # z = (h - mu) * rstd, single fused DVE pass
                nc.vector.tensor_scalar(
                    ht[rows, :], ht[rows, :],
                    stat[:][rows, 1:2], stat[:][rows, 6:7],
                    op0=mybir.AluOpType.subtract, op1=mybir.AluOpType.mult)
        sC2.close()

        # ---- transpose z -> hnT [128, 16*512] (pad cols zero) ----
        sPS2.close()
        sPS3 = ExitStack()
        pspool = sPS3.enter_context(tc.tile_pool(name="ps3", bufs=1, space="PSUM"))
        sF = ExitStack(); pF = sF.enter_context(
            tc.tile_pool(name="pF", bufs=1, side="right"))
        hnT_sb = pF.tile([128, KT * M2], F32R)
        nc.vector.tensor_copy(
            bass.AP(hnT_sb[:].tensor, hnT_sb[:].offset + W,
                    [[KT * M2, 128], [M2, KT], [64, NB], [1, 64 - W]]),
            zer_sb[:].unsqueeze(1).unsqueeze(1).broadcast_to([128, KT, NB, 64 - W]))
        for p in range(4):
            for dt in range(KT):
                tp = pspool.tile([128, 128], F32, tag=f"b{dt % 4}", name=f"tpz{p}_{dt}")
                nc.tensor.transpose(tp[:], h_tiles[p][:][:, dt * 128:(dt + 1) * 128],
                                    ident_sb[:])
                nc.vector.tensor_copy(
                    hnT_sb[:][:, dt * M2 + (2 * p) * 64: dt * M2 + (2 * p) * 64 + W],
                    tp[:][:, 0:W])
                nc.vector.tensor_copy(
                    hnT_sb[:][:, dt * M2 + (2 * p + 1) * 64: dt * M2 + (2 * p + 1) * 64 + W],
                    tp[:][:, 64:64 + W])
        sE.close(); sD.close(); sA.close()

        # ---- SA projections ----
        sG = ExitStack(); pG = sG.enter_context(tc.tile_pool(name="pG", bufs=1))
        q2T_sb = pG.tile([128, KT * M2], F32, tag="q2T")
        k2T_sb = pG.tile([128, KT * M2], F32, tag="k2T")
        _proj_T(nc, wpool, pspool, q2T_sb[:], w2qT_d, hnT_sb[:], b2q_sb[:], M2)
        _proj_T(nc, wpool, pspool, k2T_sb[:], w2kT_d, hnT_sb[:], b2k_sb[:], M2)
        sH = ExitStack(); pH = sH.enter_context(tc.tile_pool(name="pH", bufs=1))
        v2_sb = pH.tile([128, 4 * D], F32R)
        _proj_nat(nc, wpool, pspool, v2_sb[:], w2vT_d, hnT_sb[:], M2)
        sF.close()

        # ---- pooled attention ----
        sI = ExitStack(); pI = sI.enter_context(tc.tile_pool(name="pI", bufs=1))
        s2T_sb = pI.tile([128, BL], F32R, tag="s2T")
        nc.vector.tensor_copy(s2T_sb[:], zer_sb[:].broadcast_to([128, BL]))
        z2_sb = pI.tile([W, 2], F32, tag="z2")
        for b in range(BL):
            g2 = pspool.tile([64, 64], F32, tag=f"b{b % 2}", name=f"g2_{b}")
            for k in range(KT):
                nc.tensor.matmul(
                    g2[:],
                    q2T_sb[:, k * M2 + b * 64: k * M2 + b * 64 + 64],
                    k2T_sb[:, k * M2 + b * 64: k * M2 + b * 64 + 64],
                    start=(k == 0), stop=(k == KT - 1))
            e2 = pI.tile([64, 64], F32, tag=f"e2_{b % 2}", name=f"e2_{b}")
            nc.scalar.activation(e2[:], g2[:], AF.Exp, scale=SCALE)
            nc.vector.memset(e2[:][0:W, W:64], 0.0)
            zc = z2_sb[:][:, b % 2: b % 2 + 1]
            nc.vector.reduce_sum(zc, e2[:][0:W, :], axis=AX.X)
            nc.vector.reciprocal(zc, zc)
            nc.vector.tensor_scalar_mul(e2[:][0:W, :], e2[:][0:W, :], zc)
            s2p = pspool.tile([64, 1], F32, tag=f"b{2 + b % 2}", name=f"s2p{b}")
            nc.tensor.matmul(s2p[:], e2[:][0:W, :],
                             onescol[:], start=True, stop=True)
            nc.vector.tensor_copy(
                s2T_sb[:][(b % 2) * 64:(b % 2) * 64 + 64, b:b + 1], s2p[:])
        for p in range(4):
            outp = pI.tile([2, D], F32, tag=f"op{p % 2}", name=f"outp{p}")
            for c in range(4):
                op = pspool.tile([2, 512], F32, tag=f"b{4 + c}", name=f"ops{p}_{c}")
                nc.tensor.matmul(
                    op[:], s2T_sb[:][:, 2 * p: 2 * p + 2],
                    v2_sb[:][:, p * D + c * 512: p * D + (c + 1) * 512],
                    start=True, stop=True)
                nc.scalar.activation(outp[:][:, c * 512:(c + 1) * 512], op[:],
                                     AF.Identity)
            nc.sync.dma_start(out_d[2 * p: 2 * p + 2, :], outp[:])
        sI.close(); sH.close(); sG.close(); sPS3.close()

    nc.compile()
    return nc


_CACHE = {}


def _host_prep(inputs):
    f32 = lambda x: np.ascontiguousarray(np.asarray(x, dtype=np.float32))
    fc = f32(inputs["fc_feats"])
    ln_g = f32(inputs["ln_g"]); ln_b = f32(inputs["ln_b"])
    T = lambda w: np.ascontiguousarray(np.asarray(w, dtype=np.float32).T)
    t16 = lambda v: np.ascontiguousarray(
        np.asarray(v, dtype=np.float32).reshape(KT, 128).T)

    sa_wq = f32(inputs["sa_wq"]); sa_wk = f32(inputs["sa_wk"]); sa_wv = f32(inputs["sa_wv"])
    w2q = sa_wq * ln_g[None, :]
    w2k = sa_wk * ln_g[None, :]
    w2v = sa_wv * ln_g[None, :]
    b2q = f32(inputs["sa_bq"]) + sa_wq @ ln_b
    b2k = f32(inputs["sa_bk"]) + sa_wk @ ln_b
    b2v = f32(inputs["sa_bv"]) + sa_wv @ ln_b

    common = {
        "wqT": T(inputs["nsa_wq"]), "wkT": T(inputs["nsa_wk"]), "wvT": T(inputs["nsa_wv"]),
        "w2qT": T(w2q), "w2kT": T(w2k), "w2vT": T(w2v),
        "bq_t": t16(inputs["nsa_bq"]), "bk_t": t16(inputs["nsa_bk"]),
        "b2q_t": t16(b2q), "b2k_t": t16(b2k),
        "dswT": T(inputs["ds_w"]),
        "dsb": f32(inputs["ds_b"]).reshape(W, 1),
        "ident": np.eye(128, dtype=np.float32),
        "bv_aug": (8.0 * f32(inputs["nsa_bv"])).reshape(1, D),
        "onesw": np.ones((1, W), dtype=np.float32),
        "zer1": np.zeros((128, 1), dtype=np.float32),
    }
    in_maps = []
    for c in range(NCORES):
        fcc = fc[c * BL:(c + 1) * BL]
        xT = np.ascontiguousarray(fcc.reshape(M1, D).T)
        xn = np.zeros((128, 4 * D), dtype=np.float32)
        for b in range(BL):
            xn[(b % 2) * 64:(b % 2) * 64 + 64, (b // 2) * D:(b // 2 + 1) * D] = fcc[b]
        m = dict(common)
        m["xT"] = xT
        m["xn"] = xn
        in_maps.append(m)
    return in_maps, b2v


def kernel(**inputs):
    from concourse import bass_utils
    if "nc" not in _CACHE:
        _CACHE["nc"] = build_nc()
    nc = _CACHE["nc"]
    in_maps, b2v = _host_prep(inputs)
    res = bass_utils.run_bass_kernel_spmd(nc, in_maps, core_ids=list(range(NCORES)))
    out = np.concatenate([r["out"] for r in res.results], axis=0)
    out = out + (float(W) * b2v)[None, :]
    return out.astype(np.float32)
